# revision 1
# baseline (speedup 1.0000x reference)
"""GroupedQueryAttention TRN2 kernel: 8-way tensor-parallel over heads.

Sharding: core c gets query heads 4c..4c+3 (W_query rows 256c:256c+256),
KV head c (W_key/W_value rows 64c:64c+64), W_out columns 256c:256c+256.
x is replicated; each core computes a partial [T, C] output; host sums.

Per-core dataflow (all transposed "T-on-free" layout, f32r matmuls):
  xT (host-pretransposed) streamed in quarters -> q^T/k^T/v^T via PE
  RMSNorm via PE ones-matmul sumsq + ACT sqrt + DVE recip; RoPE via PE
  partition-swap matmul + DVE muls.  Attention per head: S^T strips
  [128k x 512q] (causal-trimmed), additive -60 mask on diag window, ACT
  exp into f32r P^T, A@V with ones-augmented V giving ctx^T and softmax
  sums in one accumulation.  ctx^T normalized via PE broadcast matmul,
  then out-proj back to natural [T, C] and DMA out.
"""

import sys

sys.path.insert(0, "/opt/trn_rl_repo")

import numpy as np

import concourse.bass as bass
import concourse.mybir as mybir
import concourse.tile as tile
from concourse import bacc
from concourse.bass_utils import run_bass_kernel_spmd

H, KV, D, EPS = 32, 8, 64, 1e-6
T = 2048
C = 2048
HPC = H // 8          # 4 query heads per core
DQ = HPC * D          # 256
W = 448               # qkv out dims per core (256 + 64 + 64dup + 64)
NW = 512              # matmul moving free dim
F32 = mybir.dt.float32
F32R = mybir.dt.float32r
AF = mybir.ActivationFunctionType

_PROG = None


def _build_program():
    nc = bacc.Bacc("TRN2", target_bir_lowering=False, debug=False)

    xt_d = nc.declare_dram_parameter("xt", [C, T], F32R, isOutput=False)
    wqkvt_d = nc.declare_dram_parameter("wqkvt", [C, W], F32R, isOutput=False)
    wot_d = nc.declare_dram_parameter("wot", [DQ, C], F32R, isOutput=False)
    cost_d = nc.declare_dram_parameter("cost", [128, T], F32, isOutput=False)
    sints_d = nc.declare_dram_parameter("sints", [128, T], F32, isOutput=False)
    cm_d = nc.declare_dram_parameter("cm", [128, 4 * NW], F32, isOutput=False)
    fa_d = nc.declare_dram_parameter("fa", [65, 128], F32R, isOutput=False)
    fb_d = nc.declare_dram_parameter("fb", [65, 128], F32R, isOutput=False)
    fk_d = nc.declare_dram_parameter("fk", [65, 128], F32R, isOutput=False)
    ea_d = nc.declare_dram_parameter("ea", [4, 128], F32R, isOutput=False)
    eb_d = nc.declare_dram_parameter("eb", [4, 128], F32R, isOutput=False)
    sqo_d = nc.declare_dram_parameter("sqo", [128, 5], F32R, isOutput=False)
    perm_d = nc.declare_dram_parameter("perm", [128, 128], F32R, isOutput=False)
    id64_d = nc.declare_dram_parameter("id64", [64, 64], F32R, isOutput=False)
    ones16_d = nc.declare_dram_parameter("ones16", [128, 16], F32R, isOutput=False)
    out_d = nc.declare_dram_parameter("out", [T, C], F32, isOutput=True)

    with tile.TileContext(nc) as tc:
        with tc.tile_pool(name="persist", bufs=1) as pp:
            qT = pp.tile([128, 2 * T], F32R, tag="qT")       # [dims(2x128), (m,t)]
            kkT = pp.tile([128, T], F32R, tag="kkT")         # k dup both halves
            vT = pp.tile([64, T], F32R, tag="vT")
            cosT = pp.tile([128, T], F32, tag="cosT")
            sinTs = pp.tile([128, T], F32, tag="sinTs")
            cm = pp.tile([128, 4 * NW], F32, tag="cm")
            fa = pp.tile([65, 128], F32R, tag="fa")
            fb = pp.tile([65, 128], F32R, tag="fb")
            fk = pp.tile([65, 128], F32R, tag="fk")
            ea = pp.tile([4, 128], F32R, tag="ea")
            eb = pp.tile([4, 128], F32R, tag="eb")
            sqo = pp.tile([128, 5], F32R, tag="sqo")
            perm = pp.tile([128, 128], F32R, tag="perm")
            id64 = pp.tile([64, 64], F32R, tag="id64")
            biasq = pp.tile([34, 1], F32, tag="biasq")
            biask = pp.tile([65, 1], F32, tag="biask")
            scalek = pp.tile([65, 1], F32, tag="scalek")
            nc.vector.memset(biasq[:], float(64 * EPS))
            nc.vector.memset(biask[:], float(EPS))
            nc.vector.memset(scalek[:], float(1.0 / 64))
            nc.sync.dma_start(cosT[:], cost_d[:])
            nc.sync.dma_start(sinTs[:], sints_d[:])
            nc.sync.dma_start(cm[:], cm_d[:])
            nc.sync.dma_start(fa[:], fa_d[:])
            nc.sync.dma_start(fb[:], fb_d[:])
            nc.sync.dma_start(fk[:], fk_d[:])
            nc.sync.dma_start(ea[:], ea_d[:])
            nc.sync.dma_start(eb[:], eb_d[:])
            nc.sync.dma_start(sqo[:], sqo_d[:])
            nc.sync.dma_start(perm[:], perm_d[:])
            nc.sync.dma_start(id64[:], id64_d[:])

            # ---------------- Phase 1: QKV projections ----------------
            with tc.tile_pool(name="p1sb", bufs=2) as p1sb, \
                 tc.tile_pool(name="p1w", bufs=1) as p1w, \
                 tc.tile_pool(name="p1ps", bufs=2, space="PSUM") as p1ps:
                wq = p1w.tile([128, 16 * W], F32R, tag="wq")
                nc.sync.dma_start(
                    wq[:].rearrange("p (c w) -> p c w", w=W),
                    wqkvt_d[:].rearrange("(c p) w -> p c w", p=128),
                )
                for qtr in range(4):
                    xq = p1sb.tile([128, 16 * NW], F32R, tag="xq")
                    nc.sync.dma_start(
                        xq[:].rearrange("p (c t) -> p c t", t=NW),
                        xt_d[:, qtr * NW:(qtr + 1) * NW].rearrange(
                            "(c p) t -> p c t", p=128),
                    )
                    pq0 = p1ps.tile([128, NW], F32, tag="pq0")
                    pq1 = p1ps.tile([128, NW], F32, tag="pq1")
                    pkk = p1ps.tile([128, NW], F32, tag="pkk")
                    pvv = p1ps.tile([64, NW], F32, tag="pvv")
                    for c in range(16):
                        st, sp = (c == 0), (c == 15)
                        nc.tensor.matmul(pq0[:], wq[:, W * c:W * c + 128],
                                         xq[:, NW * c:NW * (c + 1)],
                                         start=st, stop=sp)
                        nc.tensor.matmul(pq1[:], wq[:, W * c + 128:W * c + 256],
                                         xq[:, NW * c:NW * (c + 1)],
                                         start=st, stop=sp)
                        nc.tensor.matmul(pkk[:], wq[:, W * c + 256:W * c + 384],
                                         xq[:, NW * c:NW * (c + 1)],
                                         start=st, stop=sp)
                        nc.tensor.matmul(pvv[:], wq[:, W * c + 384:W * c + 448],
                                         xq[:, NW * c:NW * (c + 1)],
                                         start=st, stop=sp)
                    nc.vector.tensor_copy(qT[:, NW * qtr:NW * (qtr + 1)], pq0[:])
                    nc.vector.tensor_copy(qT[:, T + NW * qtr:T + NW * (qtr + 1)],
                                          pq1[:])
                    nc.vector.tensor_copy(kkT[:, NW * qtr:NW * (qtr + 1)], pkk[:])
                    nc.vector.tensor_copy(vT[:, NW * qtr:NW * (qtr + 1)], pvv[:])

            # ---------------- Phase 2: RMSNorm + RoPE ----------------
            with tc.tile_pool(name="p2sb", bufs=1) as p2sb, \
                 tc.tile_pool(name="p2tmp", bufs=2) as p2tmp, \
                 tc.tile_pool(name="p2ps", bufs=2, space="PSUM") as p2ps:
                rms = p2sb.tile([65, T], F32, tag="rms")
                rinv = p2sb.tile([65, T], F32, tag="rinv")
                rinvr = p2sb.tile([65, T], F32R, tag="rinvr")

                for m in range(2):
                    qc = qT[:, T * m:T * (m + 1)]
                    t2 = p2tmp.tile([128, T], F32R, tag="t2")
                    nc.vector.tensor_mul(t2[:], qc, qc)
                    ss = p2ps.tile([2, T], F32, tag="ps2")
                    for w in range(4):
                        nc.tensor.matmul(ss[:, NW * w:NW * (w + 1)],
                                         sqo[:, 2 * m:2 * m + 2],
                                         t2[:, NW * w:NW * (w + 1)],
                                         start=True, stop=True)
                    rr = 32 * m
                    nc.scalar.activation(rms[rr:rr + 2, :], ss[:],
                                         AF.Sqrt, bias=biasq[rr:rr + 2, :],
                                         scale=1.0)
                kc = kkT[:]
                t2k = p2tmp.tile([128, T], F32R, tag="t2")
                nc.vector.tensor_mul(t2k[0:64, :], kkT[0:64, :], kkT[0:64, :])
                ssk = p2ps.tile([1, T], F32, tag="ps2")
                for w in range(4):
                    nc.tensor.matmul(ssk[:, NW * w:NW * (w + 1)],
                                     sqo[0:64, 4:5],
                                     t2k[0:64, NW * w:NW * (w + 1)],
                                     start=True, stop=True)
                nc.scalar.activation(rms[64:65, :], ssk[:], AF.Sqrt,
                                     bias=biask[64:65, :],
                                     scale=scalek[64:65, :])
                for rr, n in ((0, 2), (32, 2), (64, 1)):
                    nc.vector.reciprocal(rinv[rr:rr + n, :], rms[rr:rr + n, :])
                    nc.vector.tensor_copy(rinvr[rr:rr + n, :], rinv[rr:rr + n, :])

                for m in range(2):
                    qc = qT[:, T * m:T * (m + 1)]
                    pb = p2ps.tile([128, T], F32, tag="ps2")
                    lhs = fa if m == 0 else fb
                    for w in range(4):
                        nc.tensor.matmul(pb[:, NW * w:NW * (w + 1)], lhs[:],
                                         rinvr[:, NW * w:NW * (w + 1)],
                                         start=True, stop=True)
                    nc.vector.tensor_mul(qc, qc, pb[:])
                    psw = p2ps.tile([128, T], F32, tag="ps2")
                    for w in range(4):
                        nc.tensor.matmul(psw[:, NW * w:NW * (w + 1)], perm[:],
                                         qc[:, NW * w:NW * (w + 1)],
                                         start=True, stop=True)
                    tm1 = p2tmp.tile([128, T], F32, tag="tm1")
                    tm2 = p2tmp.tile([128, T], F32, tag="tm2")
                    nc.vector.tensor_mul(tm1[:], qc, cosT[:])
                    nc.vector.tensor_mul(tm2[:], psw[:], sinTs[:])
                    nc.vector.tensor_add(qc, tm1[:], tm2[:])
                # k (duplicated in both partition halves of kkT)
                pbk = p2ps.tile([128, T], F32, tag="ps2")
                for w in range(4):
                    nc.tensor.matmul(pbk[:, NW * w:NW * (w + 1)], fk[:],
                                     rinvr[:, NW * w:NW * (w + 1)],
                                     start=True, stop=True)
                nc.vector.tensor_mul(kkT[:], kkT[:], pbk[:])
                pswk = p2ps.tile([128, T], F32, tag="ps2")
                for w in range(4):
                    nc.tensor.matmul(pswk[:, NW * w:NW * (w + 1)],
                                     perm[:],
                                     kkT[:, NW * w:NW * (w + 1)],
                                     start=True, stop=True)
                tm1k = p2tmp.tile([128, T], F32, tag="tm1")
                tm2k = p2tmp.tile([128, T], F32, tag="tm2")
                nc.vector.tensor_mul(tm1k[:], kkT[:], cosT[:])
                nc.vector.tensor_mul(tm2k[:], pswk[:], sinTs[:])
                nc.vector.tensor_add(kkT[:], tm1k[:], tm2k[:])

            # ---------------- Phase 3: V natural (ones-augmented) -------
            with tc.tile_pool(name="p3sb", bufs=1) as p3sb:
                vaug = p3sb.tile([128, 16 * 65], F32R, tag="vaug")
                nc.sync.dma_start(
                    vaug[:].rearrange("p (i c) -> p i c", c=65)[:, :, 64:65],
                    ones16_d[:].rearrange("p (i c) -> p i c", c=1),
                )
                with tc.tile_pool(name="p3ps", bufs=2, space="PSUM") as p3ps:
                    for i in range(16):
                        pv = p3ps.tile([128, 64], F32R, tag="pv")
                        nc.tensor.transpose(pv[:], vT[:, 128 * i:128 * (i + 1)],
                                            id64[:])
                        nc.vector.tensor_copy(vaug[:, 65 * i:65 * i + 64], pv[:])

                # ---------------- Phase 4: attention per head ----------
                recip = p3sb.tile([4, T], F32, tag="recip")
                sums4 = p3sb.tile([4, T], F32, tag="sums4")
                sums_sb = p3sb.tile([65, 4 * T], F32, tag="sums_sb")
                ctxT = p3sb.tile([128, 2 * T], F32R, tag="ctxT")
                tmpc = p3sb.tile([64, T], F32R, tag="tmpc")
                with tc.tile_pool(name="p4pt", bufs=2) as p4pt, \
                     tc.tile_pool(name="p4s", bufs=1, space="PSUM") as p4s, \
                     tc.tile_pool(name="p4c", bufs=1, space="PSUM") as p4c:
                    for h in range(HPC):
                        qh = qT[64 * (h % 2):64 * (h % 2) + 64,
                                T * (h // 2):T * (h // 2 + 1)]
                        ctx = p4c.tile([65, T], F32, tag="ctx")
                        for i in range(16):
                            j0 = i // 4
                            r = i % 4
                            wdt = (4 - j0) * NW
                            s_ps = p4s.tile([128, T], F32, tag="s")
                            for j in range(j0, 4):
                                nc.tensor.matmul(
                                    s_ps[:, NW * (j - j0):NW * (j - j0 + 1)],
                                    kkT[64 * (h % 2):64 * (h % 2) + 64,
                                        128 * i:128 * (i + 1)],
                                    qh[:, NW * j:NW * (j + 1)],
                                    start=True, stop=True)
                            nc.vector.tensor_add(s_ps[:, 0:NW], s_ps[:, 0:NW],
                                                 cm[:, NW * r:NW * (r + 1)])
                            pt = p4pt.tile([128, T], F32R, tag="pt")
                            nc.scalar.activation(pt[:, 0:wdt], s_ps[:, 0:wdt],
                                                 AF.Exp)
                            for j in range(j0, 4):
                                nc.tensor.matmul(
                                    ctx[:, NW * j:NW * (j + 1)],
                                    vaug[:, 65 * i:65 * (i + 1)],
                                    pt[:, NW * (j - j0):NW * (j - j0 + 1)],
                                    start=(i == 0), stop=(i == 4 * j + 3))
                        nc.vector.tensor_copy(sums_sb[64:65, T * h:T * (h + 1)],
                                               ctx[64:65, :])
                        if h % 2 == 0:
                            nc.vector.tensor_copy(
                                ctxT[0:64, T * (h // 2):T * (h // 2 + 1)],
                                ctx[0:64, :])
                        else:
                            nc.vector.tensor_copy(tmpc[:], ctx[0:64, :])
                            nc.sync.dma_start(
                                ctxT[64:128, T * (h // 2):T * (h // 2 + 1)],
                                tmpc[:])
                    for h in range(HPC):
                        nc.sync.dma_start(sums4[h:h + 1, :],
                                          sums_sb[64:65, T * h:T * (h + 1)])
                    nc.vector.reciprocal(recip[:], sums4[:])

                # ------------- Phase 5: normalize + out-proj -----------
                rinvc = p3sb.tile([4, T], F32R, tag="rinvc")
                nc.vector.tensor_copy(rinvc[:], recip[:])
                with tc.tile_pool(name="p5w", bufs=1) as p5w, \
                     tc.tile_pool(name="p5o", bufs=3) as p5o, \
                     tc.tile_pool(name="p5ps", bufs=2, space="PSUM") as p5ps:
                    wo = p5w.tile([128, 2 * T], F32R, tag="wo")
                    nc.sync.dma_start(
                        wo[:].rearrange("p (m t) -> p m t", t=T),
                        wot_d[:].rearrange("(m p) t -> p m t", p=128),
                    )
                    for m in range(2):
                        cc = ctxT[:, T * m:T * (m + 1)]
                        pb2 = p5ps.tile([128, T], F32, tag="po")
                        lhs = ea if m == 0 else eb
                        for w in range(4):
                            nc.tensor.matmul(pb2[:, NW * w:NW * (w + 1)],
                                             lhs[:], rinvc[:, NW * w:NW * (w + 1)],
                                             start=True, stop=True)
                        nc.vector.tensor_mul(cc, cc, pb2[:])
                    for t in range(16):
                        po = p5ps.tile([128, T], F32, tag="po")
                        for m2 in range(2):
                            for w in range(4):
                                nc.tensor.matmul(
                                    po[:, NW * w:NW * (w + 1)],
                                    ctxT[:, T * m2 + 128 * t:T * m2 + 128 * (t + 1)],
                                    wo[:, T * m2 + NW * w:T * m2 + NW * (w + 1)],
                                    start=(m2 == 0), stop=(m2 == 1))
                        ot = p5o.tile([128, T], F32, tag="ot")
                        nc.vector.tensor_copy(ot[:], po[:])
                        nc.sync.dma_start(out_d[128 * t:128 * (t + 1), :], ot[:])

    nc.compile()
    return nc


def _host_constants():
    iv = 1.0 / (10000.0 ** (np.arange(0, D, 2, dtype=np.float32) / D))
    ang = np.arange(T, dtype=np.float32)[:, None] * iv[None, :]
    ang = np.concatenate([ang, ang], axis=-1)          # [T, 64]
    return np.cos(ang), np.sin(ang)


def kernel(x, mask, cos, sin, W_query, W_key, W_value, W_out,
           q_norm_w, k_norm_w):
    global _PROG
    if _PROG is None:
        _PROG = _build_program()
    nc = _PROG

    x = np.asarray(x, np.float32)
    cos = np.asarray(cos, np.float32)
    sin = np.asarray(sin, np.float32)
    W_query = np.asarray(W_query, np.float32)
    W_key = np.asarray(W_key, np.float32)
    W_value = np.asarray(W_value, np.float32)
    W_out = np.asarray(W_out, np.float32)
    q_norm_w = np.asarray(q_norm_w, np.float32)
    k_norm_w = np.asarray(k_norm_w, np.float32)

    xt = np.ascontiguousarray(x[0].T)                  # [C, T]
    cosT1 = np.ascontiguousarray(cos[:T].T)            # [64, T]
    sinT1 = np.ascontiguousarray(sin[:T].T).copy()
    sinT1[0:32, :] *= -1.0                             # signed for rotate-half
    cosT = np.concatenate([cosT1, cosT1], axis=0)      # [128, T]
    sinT = np.concatenate([sinT1, sinT1], axis=0)

    # causal mask tiles for the diagonal 512-window of each k-strip
    p = np.arange(128)[:, None]
    col = np.arange(4 * NW)[None, :]
    cmf = np.zeros((128, 4 * NW), np.float32)
    for r in range(4):
        cw = col[:, NW * r:NW * (r + 1)] - NW * r
        cmf[:, NW * r:NW * (r + 1)] = np.where(cw < 128 * r + p, -60.0, 0.0)

    fa = np.zeros((65, 128), np.float32)
    fb = np.zeros((65, 128), np.float32)
    fk = np.zeros((65, 128), np.float32)
    fa[0, 0:64] = q_norm_w
    fa[1, 64:128] = q_norm_w
    fb[32, 0:64] = q_norm_w
    fb[33, 64:128] = q_norm_w
    fk[64, 0:64] = k_norm_w
    fk[64, 64:128] = k_norm_w
    ea = np.zeros((4, 128), np.float32)
    eb = np.zeros((4, 128), np.float32)
    ea[0, 0:64] = 1.0
    ea[1, 64:128] = 1.0
    eb[2, 0:64] = 1.0
    eb[3, 64:128] = 1.0
    sqo = np.zeros((128, 5), np.float32)
    sqo[0:64, 0] = 1.0
    sqo[64:128, 1] = 1.0
    sqo[0:64, 2] = 1.0
    sqo[64:128, 3] = 1.0
    sqo[0:64, 4] = 1.0
    perm = np.zeros((128, 128), np.float32)
    for b in range(2):
        for d in range(64):
            perm[64 * b + (d ^ 32), 64 * b + d] = 1.0
    id64 = np.eye(64, dtype=np.float32)

    shared = {
        "xt": xt, "cost": cosT, "sints": sinT, "cm": cmf,
        "fa": fa, "fb": fb, "fk": fk, "ea": ea, "eb": eb,
        "sqo": sqo, "perm": perm, "id64": id64,
        "ones16": np.ones((128, 16), np.float32),
    }
    in_maps = []
    for c in range(8):
        wqkvt = np.ascontiguousarray(np.concatenate(
            [W_query[DQ * c:DQ * (c + 1)],
             W_key[64 * c:64 * (c + 1)],
             W_key[64 * c:64 * (c + 1)],
             W_value[64 * c:64 * (c + 1)]], axis=0).T)   # [C, 448]
        wot = np.ascontiguousarray(W_out[:, DQ * c:DQ * (c + 1)].T)  # [256, C]
        in_maps.append(dict(shared, wqkvt=wqkvt, wot=wot))

    res = run_bass_kernel_spmd(nc, in_maps, list(range(8)))
    out = res.results[0]["out"].astype(np.float64)
    for c in range(1, 8):
        out += res.results[c]["out"]
    return out.astype(np.float32)[None]



# revision 6
# speedup vs baseline: 1.8331x; 1.8331x over previous
"""GroupedQueryAttention TRN2 kernel: 8-way tensor-parallel over heads.

Sharding: core c gets query heads 4c..4c+3 (W_query rows 256c:256c+256),
KV head c (W_key/W_value rows 64c:64c+64), W_out columns 256c:256c+256.
x is replicated; each core computes a partial [C, T] output (transposed);
host transposes and sums.

All matmul operands are bf16 (1 PE cycle/row at any p-state and free size,
half the DMA bytes); PSUM accumulation stays f32.  Per-core dataflow:
  Stage 1 (per 512-col t-quarter, software-pipelined): xT streamed in,
    QKV projections (3 matmuls per 128-contraction chunk), PSUM->SBUF
    copies on ACT/DVE, RMS sumsq via PE ones-matmul + ACT sqrt + DVE
    recip, RoPE as rope_raw(q)*bcast(rinv) with the norm weights folded
    into per-dtype cos/sin tables (exact for any q/k_norm_w), v
    transposed into ones-augmented vaug via PE.
  Stage 2 attention per (head, 1024-col q-half): causal-trimmed S strips
    at 128 granularity, exp on ACT (scale=1/8 folded in) into bf16 P,
    triangle mask as bf16 multiply post-exp, A@V with ones-augmented V
    giving ctx + softmax sums in one accumulation; 1-ahead S pipeline
    against double-buffered PSUM.  Normalize via DVE recip + PE ones
    broadcast + DVE mul into bf16 ctxT.
  Stage 3 out-proj in [C, T] orientation (PSUM = [128 c-feat, 512 t]),
    copies alternate ACT/DVE, bf16 DMA out.
"""

import sys

sys.path.insert(0, "/opt/trn_rl_repo")

import numpy as np
import ml_dtypes

import concourse.bass as bass
import concourse.mybir as mybir
import concourse.tile as tile
from concourse import bacc
from concourse.bass_utils import run_bass_kernel_spmd

H, KV, D, EPS = 32, 8, 64, 1e-6
T = 2048
C = 2048
DQ = 256              # q out dims per core
NW = 512
F32 = mybir.dt.float32
BF16 = mybir.dt.bfloat16
AF = mybir.ActivationFunctionType
BF = ml_dtypes.bfloat16

_PROG = None


def _build_program():
    nc = bacc.Bacc("TRN2", target_bir_lowering=False, debug=False)

    xt_d = nc.declare_dram_parameter("xt", [C, T], BF16, isOutput=False)
    wq_d = nc.declare_dram_parameter("wq", [128, 16 * 384], BF16, isOutput=False)
    wo_d = nc.declare_dram_parameter("wo", [128, 2 * T], BF16, isOutput=False)
    cosq_d = nc.declare_dram_parameter("cosq", [128, T], BF16, isOutput=False)
    sinq_d = nc.declare_dram_parameter("sinq", [128, T], BF16, isOutput=False)
    cosk_d = nc.declare_dram_parameter("cosk", [64, T], BF16, isOutput=False)
    sink_d = nc.declare_dram_parameter("sink", [64, T], BF16, isOutput=False)
    tri_d = nc.declare_dram_parameter("tri", [128, 128], BF16, isOutput=False)
    sqo_d = nc.declare_dram_parameter("sqo", [128, 2], BF16, isOutput=False)
    perm_d = nc.declare_dram_parameter("perm", [128, 128], BF16, isOutput=False)
    id64_d = nc.declare_dram_parameter("id64", [64, 64], BF16, isOutput=False)
    onesv_d = nc.declare_dram_parameter("onesv", [128, 16], BF16, isOutput=False)
    bsel_d = nc.declare_dram_parameter("bsel", [65, 320], BF16, isOutput=False)
    bone_d = nc.declare_dram_parameter("bone", [1, 64], BF16, isOutput=False)
    out_d = nc.declare_dram_parameter("out", [C, T], BF16, isOutput=True)

    with tile.TileContext(nc) as tc:
        with tc.tile_pool(name="persist", bufs=1) as pp:
            qT = pp.tile([128, 2 * T], BF16, tag="qT")
            kkT = pp.tile([128, T], BF16, tag="kkT")
            vaug = pp.tile([128, 16 * 65], BF16, tag="vaug")
            ctxT = pp.tile([128, 2 * T], BF16, tag="ctxT")
            cosq = pp.tile([128, T], BF16, tag="cosq")
            sinq = pp.tile([128, T], BF16, tag="sinq")
            cosk = pp.tile([64, T], BF16, tag="cosk")
            sink = pp.tile([64, T], BF16, tag="sink")
            wq = pp.tile([128, 16 * 384], BF16, tag="wq")
            wo = pp.tile([128, 2 * T], BF16, tag="wo")
            tri = pp.tile([128, 128], BF16, tag="tri")
            sqo = pp.tile([128, 2], BF16, tag="sqo")
            perm = pp.tile([128, 128], BF16, tag="perm")
            id64 = pp.tile([64, 64], BF16, tag="id64")
            bsel = pp.tile([65, 320], BF16, tag="bsel")
            bone = pp.tile([1, 64], BF16, tag="bone")
            epsb = pp.tile([65, 1], F32, tag="epsb")
            nc.vector.memset(epsb[:], float(EPS))
            for t_, d_ in ((cosq, cosq_d), (sinq, sinq_d), (cosk, cosk_d),
                           (sink, sink_d), (tri, tri_d), (sqo, sqo_d),
                           (perm, perm_d), (id64, id64_d), (bsel, bsel_d),
                           (bone, bone_d), (wq, wq_d), (wo, wo_d)):
                nc.sync.dma_start(t_[:], d_[:])
            nc.sync.dma_start(
                vaug[:].rearrange("p (i c) -> p i c", c=65)[:, :, 64:65],
                onesv_d[:].rearrange("p (i c) -> p i c", c=1),
            )

            # ---------------- Stage 1: QKV + RMSNorm + RoPE ----------------
            with tc.tile_pool(name="s1x", bufs=2) as s1x, \
                 tc.tile_pool(name="s1s", bufs=2) as s1s, \
                 tc.tile_pool(name="s1pq", bufs=2, space="PSUM") as s1pq, \
                 tc.tile_pool(name="s1sm", bufs=2, space="PSUM") as s1sm:

                def qkv(qtr):
                    xq = s1x.tile([128, 16 * NW], BF16, tag="xq")
                    nc.sync.dma_start(
                        xq[:].rearrange("p (c t) -> p c t", t=NW),
                        xt_d[:, qtr * NW:(qtr + 1) * NW].rearrange(
                            "(c p) t -> p c t", p=128),
                    )
                    pq0 = s1pq.tile([128, NW], F32, tag="pq0")
                    pq1 = s1pq.tile([128, NW], F32, tag="pq1")
                    pkv = s1pq.tile([128, NW], F32, tag="pkv")
                    for ci in range(16):
                        st, sp = ci == 0, ci == 15
                        xc = xq[:, NW * ci:NW * (ci + 1)]
                        nc.tensor.matmul(pq0[:], wq[:, 384 * ci:384 * ci + 128],
                                         xc, start=st, stop=sp)
                        nc.tensor.matmul(pq1[:],
                                         wq[:, 384 * ci + 128:384 * ci + 256],
                                         xc, start=st, stop=sp)
                        nc.tensor.matmul(pkv[:],
                                         wq[:, 384 * ci + 256:384 * ci + 384],
                                         xc, start=st, stop=sp)
                    return pq0, pq1, pkv

                def post(qtr, pq0, pq1, pkv):
                    w0 = qtr * NW
                    qraw0 = s1s.tile([128, NW], BF16, tag="qraw0")
                    qraw1 = s1s.tile([128, NW], BF16, tag="qraw1")
                    kraw = s1s.tile([64, NW], BF16, tag="kraw")
                    vraw = s1s.tile([64, NW], BF16, tag="vraw")
                    nc.scalar.copy(qraw0[:], pq0[:])
                    nc.scalar.copy(qraw1[:], pq1[:])
                    nc.vector.tensor_copy(kraw[:], pkv[0:64, :])
                    nc.vector.tensor_copy(vraw[:], pkv[64:128, :])
                    # v -> vaug (transposed, ones-augmented)
                    for j in range(4):
                        ii = (w0 // 128) + j
                        pv = s1sm.tile([128, 64], BF16, tag="sm")
                        nc.tensor.transpose(pv[:], vraw[:, 128 * j:128 * (j + 1)],
                                            id64[:])
                        nc.vector.tensor_copy(vaug[:, 65 * ii:65 * ii + 64],
                                              pv[:])
                    # RMS sumsq -> sqrt -> recip
                    t20 = s1s.tile([128, NW], BF16, tag="t20")
                    t21 = s1s.tile([128, NW], BF16, tag="t21")
                    t2k = s1s.tile([64, NW], BF16, tag="t2k")
                    nc.vector.tensor_mul(t20[:], qraw0[:], qraw0[:])
                    nc.vector.tensor_mul(t21[:], qraw1[:], qraw1[:])
                    nc.vector.tensor_mul(t2k[:], kraw[:], kraw[:])
                    srow = s1sm.tile([65, NW], F32, tag="sm")
                    nc.tensor.matmul(srow[0:2, :], sqo[:, 0:2], t20[:],
                                     start=True, stop=True)
                    nc.tensor.matmul(srow[32:34, :], sqo[:, 0:2], t21[:],
                                     start=True, stop=True)
                    nc.tensor.matmul(srow[64:65, :], sqo[0:64, 0:1], t2k[:],
                                     start=True, stop=True)
                    rms5 = s1s.tile([65, NW], F32, tag="rms5")
                    for lo, hi in ((0, 2), (32, 34), (64, 65)):
                        nc.scalar.activation(rms5[lo:hi, :], srow[lo:hi, :],
                                             AF.Sqrt, bias=epsb[lo:hi, :],
                                             scale=1.0 / 64)
                    rb5 = s1s.tile([65, NW], BF16, tag="rb5")
                    with nc.allow_low_precision(reason="rmsnorm recip bf16"):
                        for lo, hi in ((0, 2), (32, 34), (64, 65)):
                            nc.vector.reciprocal(rb5[lo:hi, :], rms5[lo:hi, :])
                    # rinv broadcast per group (PE one-hot) + ACT copy to SBUF
                    bbs = []
                    for g, (plo, phi, lo, hi, npart) in enumerate(
                            ((0, 2, 0, 128, 128), (32, 34, 128, 256, 128),
                             (64, 65, 256, 320, 64))):
                        bb = s1sm.tile([128, NW], F32, tag="sm")
                        nc.tensor.matmul(bb[0:npart, :], bsel[plo:phi, lo:hi],
                                         rb5[plo:phi, :], start=True, stop=True)
                        bs = s1s.tile([128, NW], BF16, tag=f"bbs{g}")
                        nc.scalar.copy(bs[0:npart, :], bb[0:npart, :])
                        bbs.append(bs)
                    # RoPE: rope_raw(q) * bcast(rinv); norm weights live in
                    # the cos/sin tables
                    for g, qr in enumerate((qraw0, qraw1)):
                        ppm = s1sm.tile([128, NW], F32, tag="sm")
                        nc.tensor.matmul(ppm[:], perm[:], qr[:],
                                         start=True, stop=True)
                        tm1 = s1s.tile([128, NW], BF16, tag="tm1")
                        tm2 = s1s.tile([128, NW], BF16, tag="tm2")
                        tsum = s1s.tile([128, NW], BF16, tag="tsum")
                        nc.vector.tensor_mul(tm1[:], qr[:],
                                             cosq[:, w0:w0 + NW])
                        nc.vector.tensor_mul(tm2[:], ppm[:],
                                             sinq[:, w0:w0 + NW])
                        nc.vector.tensor_add(tsum[:], tm1[:], tm2[:])
                        nc.vector.tensor_mul(qT[:, g * T + w0:g * T + w0 + NW],
                                             tsum[:], bbs[g][:])
                    ppk = s1sm.tile([128, NW], F32, tag="sm")
                    nc.tensor.matmul(ppk[0:64, :], perm[0:64, 0:64], kraw[:],
                                     start=True, stop=True)
                    tk1 = s1s.tile([64, NW], BF16, tag="tk1")
                    tk2 = s1s.tile([64, NW], BF16, tag="tk2")
                    tks = s1s.tile([64, NW], BF16, tag="tks")
                    nc.vector.tensor_mul(tk1[:], kraw[:], cosk[:, w0:w0 + NW])
                    nc.vector.tensor_mul(tk2[:], ppk[0:64, :],
                                         sink[:, w0:w0 + NW])
                    nc.vector.tensor_add(tks[:], tk1[:], tk2[:])
                    nc.vector.tensor_mul(kkT[0:64, w0:w0 + NW], tks[:],
                                         bbs[2][0:64, :])
                    nc.vector.tensor_copy(kkT[64:128, w0:w0 + NW],
                                          kkT[0:64, w0:w0 + NW])

                prev = None
                for qtr in range(4):
                    cur = qkv(qtr)
                    if prev is not None:
                        post(qtr - 1, *prev)
                    prev = cur
                post(3, *prev)

            # ---------------- Stage 2: attention ----------------
            with tc.tile_pool(name="actx", bufs=2, space="PSUM") as actx, \
                 tc.tile_pool(name="asp", bufs=2, space="PSUM") as asp, \
                 tc.tile_pool(name="aptp", bufs=3) as aptp, \
                 tc.tile_pool(name="asb", bufs=2) as asb:
                for h in range(4):
                    sub, m = h % 2, h // 2
                    qrow = 64 * sub
                    for half in range(2):
                        base = 1024 * half
                        nstrips = 8 if half == 0 else 16
                        ctx = actx.tile([65, 1024], F32, tag="ctx")

                        def s_of(i):
                            col0 = max(128 * i - base, 0)
                            swin = (col0 // 512) * 512
                            r = (col0 - swin) // 128
                            wdt = 1024 - swin
                            s = asp.tile([128, 1024], F32, tag="s")
                            c = 128 * r
                            while c < wdt:
                                ce = min((c // 512 + 1) * 512, wdt)
                                nc.tensor.matmul(
                                    s[:, c:ce],
                                    kkT[qrow:qrow + 64, 128 * i:128 * (i + 1)],
                                    qT[qrow:qrow + 64,
                                       m * T + base + swin + c:
                                       m * T + base + swin + ce],
                                    start=True, stop=True)
                                c = ce
                            return s, swin, r, wdt

                        pend = s_of(0)
                        for i in range(nstrips):
                            s, swin, r, wdt = pend
                            pt = aptp.tile([128, 1024], BF16, tag="pt")
                            nc.scalar.activation(pt[:, 128 * r:wdt],
                                                 s[:, 128 * r:wdt],
                                                 AF.Exp, scale=0.125)
                            if r > 0:
                                nc.vector.memset(pt[:, 0:128 * r], 0.0)
                            if 128 * i >= base:
                                nc.vector.tensor_mul(
                                    pt[:, 128 * r:128 * r + 128],
                                    pt[:, 128 * r:128 * r + 128], tri[:])
                            if i + 1 < nstrips:
                                pend = s_of(i + 1)
                            for j in range(swin // 512, 2):
                                last = 4 * j + 3 if half == 0 else 4 * j + 11
                                nc.tensor.matmul(
                                    ctx[:, 512 * j:512 * (j + 1)],
                                    vaug[:, 65 * i:65 * (i + 1)],
                                    pt[:, 512 * j - swin:512 * j - swin + 512],
                                    start=(i == 0), stop=(i == last))
                        # normalize: recip -> PE broadcast -> DVE mul
                        rb = asb.tile([1, 1024], BF16, tag="rb")
                        with nc.allow_low_precision(reason="softmax recip"):
                            nc.vector.reciprocal(rb[:], ctx[64:65, :])
                        rbbp = asp.tile([64, 1024], F32, tag="s")
                        for c in (0, 512):
                            nc.tensor.matmul(rbbp[:, c:c + 512], bone[:],
                                             rb[:, c:c + 512],
                                             start=True, stop=True)
                        rbbs = asb.tile([64, 1024], BF16, tag="rbbs")
                        nc.vector.tensor_copy(rbbs[:], rbbp[:])
                        nc.vector.tensor_mul(
                            ctxT[qrow:qrow + 64,
                                 m * T + base:m * T + base + 1024],
                            ctx[0:64, :], rbbs[:])

            # ---------------- Stage 3: out-projection ----------------
            with tc.tile_pool(name="ops", bufs=4, space="PSUM") as ops, \
                 tc.tile_pool(name="osb", bufs=4) as osb:
                for tw in range(4):
                    for cb in range(16):
                        po = ops.tile([128, 512], F32, tag="po")
                        for mm in range(2):
                            nc.tensor.matmul(
                                po[:],
                                wo[:, T * mm + 128 * cb:T * mm + 128 * (cb + 1)],
                                ctxT[:, T * mm + 512 * tw:T * mm + 512 * (tw + 1)],
                                start=(mm == 0), stop=(mm == 1))
                        ob = osb.tile([128, 512], BF16, tag="ob")
                        if cb % 2 == 0:
                            nc.scalar.copy(ob[:], po[:])
                        else:
                            nc.vector.tensor_copy(ob[:], po[:])
                        nc.sync.dma_start(
                            out_d[128 * cb:128 * (cb + 1),
                                  512 * tw:512 * (tw + 1)], ob[:])

    nc.compile()
    return nc


def kernel(x, mask, cos, sin, W_query, W_key, W_value, W_out,
           q_norm_w, k_norm_w):
    global _PROG
    if _PROG is None:
        _PROG = _build_program()
    nc = _PROG

    x = np.asarray(x, np.float32)
    cos = np.asarray(cos, np.float32)
    sin = np.asarray(sin, np.float32)
    W_query = np.asarray(W_query, np.float32)
    W_key = np.asarray(W_key, np.float32)
    W_value = np.asarray(W_value, np.float32)
    W_out = np.asarray(W_out, np.float32)
    q_norm_w = np.asarray(q_norm_w, np.float32)
    k_norm_w = np.asarray(k_norm_w, np.float32)

    xt = np.ascontiguousarray(x[0].T).astype(BF)            # [C, T]

    # RoPE tables with the RMSNorm weights folded in:
    #   rope(q * w)[d] = q[d] w[d] cos[d] + sign[d] q[d^32] w[d^32] sin[d]
    cos1 = cos[:T].T.astype(np.float32)                     # [64, T]
    sin1 = sin[:T].T.astype(np.float32).copy()
    sgn = np.where((np.arange(D) % 64) < 32, -1.0, 1.0).astype(np.float32)
    dperm = np.arange(D) ^ 32
    cq1 = cos1 * q_norm_w[:, None]
    sq1 = sin1 * sgn[:, None] * q_norm_w[dperm][:, None]
    cosq = np.ascontiguousarray(np.concatenate([cq1, cq1], 0)).astype(BF)
    sinq = np.ascontiguousarray(np.concatenate([sq1, sq1], 0)).astype(BF)
    cosk = np.ascontiguousarray(cos1 * k_norm_w[:, None]).astype(BF)
    sink = np.ascontiguousarray(
        sin1 * sgn[:, None] * k_norm_w[dperm][:, None]).astype(BF)

    p = np.arange(128)[:, None]
    j = np.arange(128)[None, :]
    tri = np.where(p > j, 0.0, 1.0).astype(BF)              # S^T triangle

    sqo = np.zeros((128, 2), np.float32)
    sqo[0:64, 0] = 1.0
    sqo[64:128, 1] = 1.0
    perm = np.zeros((128, 128), np.float32)
    for b in range(2):
        for d_ in range(64):
            perm[64 * b + (d_ ^ 32), 64 * b + d_] = 1.0
    bsel = np.zeros((65, 320), np.float32)
    bsel[0, 0:64] = 1.0
    bsel[1, 64:128] = 1.0
    bsel[32, 128 + 0:128 + 64] = 1.0
    bsel[33, 128 + 64:128 + 128] = 1.0
    bsel[64, 256:320] = 1.0

    shared = {
        "xt": xt, "cosq": cosq, "sinq": sinq, "cosk": cosk, "sink": sink,
        "tri": tri, "sqo": sqo.astype(BF), "perm": perm.astype(BF),
        "id64": np.eye(64, dtype=np.float32).astype(BF),
        "onesv": np.ones((128, 16), np.float32).astype(BF),
        "bsel": bsel.astype(BF), "bone": np.ones((1, 64), np.float32).astype(BF),
    }
    in_maps = []
    for c in range(8):
        wqkv = np.concatenate(
            [W_query[DQ * c:DQ * (c + 1)],
             W_key[64 * c:64 * (c + 1)],
             W_value[64 * c:64 * (c + 1)]], axis=0).T       # [C, 384]
        wq_h = np.ascontiguousarray(
            wqkv.reshape(16, 128, 384).transpose(1, 0, 2).reshape(128, 6144)
        ).astype(BF)
        wo_sl = W_out[:, DQ * c:DQ * (c + 1)].T             # [256, C]
        wo_h = np.ascontiguousarray(
            wo_sl.reshape(2, 128, C).transpose(1, 0, 2).reshape(128, 2 * C)
        ).astype(BF)
        in_maps.append(dict(shared, wq=wq_h, wo=wo_h))

    res = run_bass_kernel_spmd(nc, in_maps, list(range(8)))
    acc = np.zeros((C, T), np.float32)
    for c in range(8):
        acc += res.results[c]["out"].astype(np.float32)
    return np.ascontiguousarray(acc.T)[None]


# revision 8
# speedup vs baseline: 2.1073x; 1.1496x over previous
"""GroupedQueryAttention TRN2 kernel: 8-way tensor-parallel over heads.

Sharding: core c gets query heads 4c..4c+3 (W_query rows 256c:256c+256),
KV head c (W_key/W_value rows 64c:64c+64), W_out columns 256c:256c+256.
x is replicated; each core computes a partial [C, T] output (transposed);
host transposes and sums.

All matmul operands are bf16 (1 PE cycle/row at any p-state and free size,
half the DMA bytes); PSUM accumulation stays f32.  Per-core dataflow:
  Stage 1 (per 512-col t-quarter, software-pipelined): xT streamed in,
    QKV projections (3 matmuls per 128-contraction chunk), PSUM->SBUF
    copies on ACT/DVE, RMS sumsq via PE ones-matmul + ACT sqrt + DVE
    recip, RoPE as rope_raw(q)*bcast(rinv) with the norm weights folded
    into per-dtype cos/sin tables (exact for any q/k_norm_w), v
    transposed into ones-augmented vaug via PE.
  Stage 2 attention per (head, 1024-col q-half): causal-trimmed S strips
    at 128 granularity, exp on ACT (scale=1/8 folded in) into bf16 P,
    triangle mask as bf16 multiply post-exp, A@V with ones-augmented V
    giving ctx + softmax sums in one accumulation; 1-ahead S pipeline
    against double-buffered PSUM.  Normalize via DVE recip + PE ones
    broadcast + DVE mul into bf16 ctxT.
  Stage 3 out-proj in [C, T] orientation (PSUM = [128 c-feat, 512 t]),
    copies alternate ACT/DVE, bf16 DMA out.
"""

import sys

sys.path.insert(0, "/opt/trn_rl_repo")

import numpy as np
import ml_dtypes

import concourse.bass as bass
import concourse.mybir as mybir
import concourse.tile as tile
from concourse import bacc
from concourse.bass_utils import run_bass_kernel_spmd

H, KV, D, EPS = 32, 8, 64, 1e-6
T = 2048
C = 2048
DQ = 256              # q out dims per core
NW = 512
F32 = mybir.dt.float32
BF16 = mybir.dt.bfloat16
AF = mybir.ActivationFunctionType
BF = ml_dtypes.bfloat16

_PROG = None


def _build_program():
    nc = bacc.Bacc("TRN2", target_bir_lowering=False, debug=False)

    xt_d = nc.declare_dram_parameter("xt", [C, T], BF16, isOutput=False)
    wq_d = nc.declare_dram_parameter("wq", [128, 16 * 384], BF16, isOutput=False)
    wo_d = nc.declare_dram_parameter("wo", [128, 2 * T], BF16, isOutput=False)
    cosq_d = nc.declare_dram_parameter("cosq", [128, T], BF16, isOutput=False)
    sinq_d = nc.declare_dram_parameter("sinq", [128, T], BF16, isOutput=False)
    cosk_d = nc.declare_dram_parameter("cosk", [64, T], BF16, isOutput=False)
    sink_d = nc.declare_dram_parameter("sink", [64, T], BF16, isOutput=False)
    tri_d = nc.declare_dram_parameter("tri", [128, 128], BF16, isOutput=False)
    sqo_d = nc.declare_dram_parameter("sqo", [128, 2], BF16, isOutput=False)
    perm_d = nc.declare_dram_parameter("perm", [128, 128], BF16, isOutput=False)
    id64_d = nc.declare_dram_parameter("id64", [64, 64], BF16, isOutput=False)
    onesv_d = nc.declare_dram_parameter("onesv", [128, 16], BF16, isOutput=False)
    bsel_d = nc.declare_dram_parameter("bsel", [65, 320], BF16, isOutput=False)
    bone_d = nc.declare_dram_parameter("bone", [1, 64], BF16, isOutput=False)
    out_d = nc.declare_dram_parameter("out", [C, T], BF16, isOutput=True)

    with tile.TileContext(nc) as tc:
        with tc.tile_pool(name="persist", bufs=1) as pp:
            qT = pp.tile([128, 2 * T], BF16, tag="qT")
            kkT = pp.tile([128, T], BF16, tag="kkT")
            vaug = pp.tile([128, 16 * 65], BF16, tag="vaug")
            ctxT = pp.tile([128, 2 * T], BF16, tag="ctxT")
            cosq = pp.tile([128, T], BF16, tag="cosq")
            sinq = pp.tile([128, T], BF16, tag="sinq")
            cosk = pp.tile([64, T], BF16, tag="cosk")
            sink = pp.tile([64, T], BF16, tag="sink")
            wq = pp.tile([128, 16 * 384], BF16, tag="wq")
            wo = pp.tile([128, 2 * T], BF16, tag="wo")
            tri = pp.tile([128, 128], BF16, tag="tri")
            sqo = pp.tile([128, 2], BF16, tag="sqo")
            perm = pp.tile([128, 128], BF16, tag="perm")
            id64 = pp.tile([64, 64], BF16, tag="id64")
            bsel = pp.tile([65, 320], BF16, tag="bsel")
            bone = pp.tile([1, 64], BF16, tag="bone")
            epsb = pp.tile([65, 1], F32, tag="epsb")
            nc.vector.memset(epsb[:], float(EPS))
            # first ACT op is a Sqrt so the initial table load picks the
            # sqrt set (stage-1 Copy lives there too); the only other load
            # is the exp set at attention (tail Copy reuses it).
            dsq = pp.tile([1, 1], F32, tag="dsq")
            nc.scalar.activation(dsq[:], epsb[0:1, :], AF.Sqrt,
                                 bias=epsb[0:1, :], scale=1.0)

            # ---------------- Stage 1: QKV + RMSNorm + RoPE ----------------
            with tc.tile_pool(name="s1x", bufs=2) as s1x, \
                 tc.tile_pool(name="s1s", bufs=2) as s1s, \
                 tc.tile_pool(name="s1pq", bufs=2, space="PSUM") as s1pq, \
                 tc.tile_pool(name="s1sm", bufs=2, space="PSUM") as s1sm:

                def xload(qtr):
                    xq = s1x.tile([128, 16 * NW], BF16, tag="xq")
                    src = xt_d[:, qtr * NW:(qtr + 1) * NW].rearrange(
                        "(c p) t -> p c t", p=128)
                    dst = xq[:].rearrange("p (c t) -> p c t", t=NW)
                    for g_ in range(4):
                        nc.sync.dma_start(dst[:, 4 * g_:4 * (g_ + 1), :],
                                          src[:, 4 * g_:4 * (g_ + 1), :])
                    return xq

                def qkv(qtr, xq=None):
                    if xq is None:
                        xq = xload(qtr)
                    pq0 = s1pq.tile([128, NW], F32, tag="pq0")
                    pq1 = s1pq.tile([128, NW], F32, tag="pq1")
                    pkv = s1pq.tile([128, NW], F32, tag="pkv")
                    for ci in range(16):
                        st, sp = ci == 0, ci == 15
                        xc = xq[:, NW * ci:NW * (ci + 1)]
                        nc.tensor.matmul(pq0[:], wq[:, 384 * ci:384 * ci + 128],
                                         xc, start=st, stop=sp)
                        nc.tensor.matmul(pq1[:],
                                         wq[:, 384 * ci + 128:384 * ci + 256],
                                         xc, start=st, stop=sp)
                        nc.tensor.matmul(pkv[:],
                                         wq[:, 384 * ci + 256:384 * ci + 384],
                                         xc, start=st, stop=sp)
                    return pq0, pq1, pkv

                def post(qtr, pq0, pq1, pkv):
                    w0 = qtr * NW
                    qraw0 = s1s.tile([128, NW], BF16, tag="qraw0")
                    qraw1 = s1s.tile([128, NW], BF16, tag="qraw1")
                    kraw = s1s.tile([64, NW], BF16, tag="kraw")
                    vraw = s1s.tile([64, NW], BF16, tag="vraw")
                    nc.scalar.copy(qraw0[:], pq0[:])
                    nc.scalar.copy(qraw1[:], pq1[:])
                    nc.vector.tensor_copy(kraw[:], pkv[0:64, :])
                    nc.vector.tensor_copy(vraw[:], pkv[64:128, :])
                    # PE: perm matmuls first (only need raw copies)
                    ppm0 = s1sm.tile([128, NW], F32, tag="sm")
                    nc.tensor.matmul(ppm0[:], perm[:], qraw0[:],
                                     start=True, stop=True)
                    ppm1 = s1sm.tile([128, NW], F32, tag="sm")
                    nc.tensor.matmul(ppm1[:], perm[:], qraw1[:],
                                     start=True, stop=True)
                    ppk = s1sm.tile([128, NW], F32, tag="sm")
                    nc.tensor.matmul(ppk[0:64, :], perm[0:64, 0:64], kraw[:],
                                     start=True, stop=True)
                    # RMS sumsq
                    t20 = s1s.tile([128, NW], BF16, tag="t20")
                    t21 = s1s.tile([128, NW], BF16, tag="t21")
                    t2k = s1s.tile([64, NW], BF16, tag="t2k")
                    nc.vector.tensor_mul(t20[:], qraw0[:], qraw0[:])
                    nc.vector.tensor_mul(t21[:], qraw1[:], qraw1[:])
                    nc.vector.tensor_mul(t2k[:], kraw[:], kraw[:])
                    srow = s1sm.tile([65, NW], F32, tag="sm")
                    nc.tensor.matmul(srow[0:2, :], sqo[:, 0:2], t20[:],
                                     start=True, stop=True)
                    nc.tensor.matmul(srow[32:34, :], sqo[:, 0:2], t21[:],
                                     start=True, stop=True)
                    nc.tensor.matmul(srow[64:65, :], sqo[0:64, 0:1], t2k[:],
                                     start=True, stop=True)
                    # RoPE partials that don't need rinv
                    tm1_0 = s1s.tile([128, NW], BF16, tag="tm1_0")
                    tm1_1 = s1s.tile([128, NW], BF16, tag="tm1_1")
                    tk1 = s1s.tile([64, NW], BF16, tag="tk1")
                    nc.vector.tensor_mul(tm1_0[:], qraw0[:], cosq[:, w0:w0 + NW])
                    nc.vector.tensor_mul(tm1_1[:], qraw1[:], cosq[:, w0:w0 + NW])
                    nc.vector.tensor_mul(tk1[:], kraw[:], cosk[:, w0:w0 + NW])
                    tsum0 = s1s.tile([128, NW], BF16, tag="tsum0")
                    tsum1 = s1s.tile([128, NW], BF16, tag="tsum1")
                    tks = s1s.tile([64, NW], BF16, tag="tks")
                    nc.vector.tensor_mul(tsum0[:], ppm0[:], sinq[:, w0:w0 + NW])
                    nc.vector.tensor_add(tsum0[:], tsum0[:], tm1_0[:])
                    nc.vector.tensor_mul(tsum1[:], ppm1[:], sinq[:, w0:w0 + NW])
                    nc.vector.tensor_add(tsum1[:], tsum1[:], tm1_1[:])
                    nc.vector.tensor_mul(tks[:], ppk[0:64, :], sink[:, w0:w0 + NW])
                    nc.vector.tensor_add(tks[:], tks[:], tk1[:])
                    # v -> vaug (transposed, ones-augmented)
                    for j in range(4):
                        ii = (w0 // 128) + j
                        pv = s1sm.tile([128, 64], BF16, tag="sm")
                        nc.tensor.transpose(pv[:], vraw[:, 128 * j:128 * (j + 1)],
                                            id64[:])
                        nc.vector.tensor_copy(vaug[:, 65 * ii:65 * ii + 64],
                                              pv[:])
                    # sqrt -> recip -> broadcast
                    rms5 = s1s.tile([65, NW], F32, tag="rms5")
                    for lo, hi in ((0, 2), (32, 34), (64, 65)):
                        nc.scalar.activation(rms5[lo:hi, :], srow[lo:hi, :],
                                             AF.Sqrt, bias=epsb[lo:hi, :],
                                             scale=1.0 / 64)
                    rb5 = s1s.tile([65, NW], BF16, tag="rb5")
                    with nc.allow_low_precision(reason="rmsnorm recip bf16"):
                        for lo, hi in ((0, 2), (32, 34), (64, 65)):
                            nc.vector.reciprocal(rb5[lo:hi, :], rms5[lo:hi, :])
                    bbs = []
                    for g, (plo, phi, lo, hi, npart) in enumerate(
                            ((0, 2, 0, 128, 128), (32, 34, 128, 256, 128),
                             (64, 65, 256, 320, 64))):
                        bb = s1sm.tile([128, NW], F32, tag="sm")
                        nc.tensor.matmul(bb[0:npart, :], bsel[plo:phi, lo:hi],
                                         rb5[plo:phi, :], start=True, stop=True)
                        bs = s1s.tile([128, NW], BF16, tag=f"bbs{g}")
                        nc.scalar.copy(bs[0:npart, :], bb[0:npart, :])
                        bbs.append(bs)
                    nc.vector.tensor_mul(qT[:, 0 * T + w0:0 * T + w0 + NW],
                                         tsum0[:], bbs[0][:])
                    nc.vector.tensor_mul(qT[:, 1 * T + w0:1 * T + w0 + NW],
                                         tsum1[:], bbs[1][:])
                    nc.vector.tensor_mul(kkT[0:64, w0:w0 + NW], tks[:],
                                         bbs[2][0:64, :])
                    nc.vector.tensor_copy(kkT[64:128, w0:w0 + NW],
                                          kkT[0:64, w0:w0 + NW])

                xq0 = xload(0)
                for g_ in range(4):
                    nc.sync.dma_start(wq[:, 1536 * g_:1536 * (g_ + 1)],
                                      wq_d[:, 1536 * g_:1536 * (g_ + 1)])
                cur = qkv(0, xq0)
                xq1 = xload(1)
                for t_, d_ in ((cosq, cosq_d), (sinq, sinq_d), (cosk, cosk_d),
                               (sink, sink_d), (sqo, sqo_d), (perm, perm_d),
                               (id64, id64_d), (bsel, bsel_d), (tri, tri_d),
                               (bone, bone_d)):
                    nc.sync.dma_start(t_[:], d_[:])
                nc.sync.dma_start(
                    vaug[:].rearrange("p (i c) -> p i c", c=65)[:, :, 64:65],
                    onesv_d[:].rearrange("p (i c) -> p i c", c=1),
                )
                prev = cur
                cur = qkv(1, xq1)
                post(0, *prev)
                prev = cur
                cur = qkv(2)
                post(1, *prev)
                prev = cur
                cur = qkv(3)
                post(2, *prev)
                nc.sync.dma_start(wo[:], wo_d[:])
                post(3, *cur)

            # ---------------- Stage 2: attention ----------------
            with tc.tile_pool(name="actx", bufs=2, space="PSUM") as actx, \
                 tc.tile_pool(name="asp", bufs=2, space="PSUM") as asp, \
                 tc.tile_pool(name="aptp", bufs=3) as aptp, \
                 tc.tile_pool(name="asb", bufs=2) as asb:

                def s_of(i, qrow, m, base):
                    col0 = max(128 * i - base, 0)
                    swin = (col0 // 512) * 512
                    r = (col0 - swin) // 128
                    wdt = 1024 - swin
                    st = asp.tile([128, 1024], F32, tag="s")
                    c = 128 * r
                    while c < wdt:
                        ce = min((c // 512 + 1) * 512, wdt)
                        nc.tensor.matmul(
                            st[:, c:ce],
                            kkT[qrow:qrow + 64, 128 * i:128 * (i + 1)],
                            qT[qrow:qrow + 64,
                               m * T + base + swin + c:m * T + base + swin + ce],
                            start=True, stop=True)
                        c = ce
                    return st, swin, r, wdt

                def do_norm(ctx, qrow, m, base):
                    rb = asb.tile([1, 1024], BF16, tag="rb")
                    with nc.allow_low_precision(reason="softmax recip"):
                        nc.vector.reciprocal(rb[:], ctx[64:65, :])
                    rbbp = asp.tile([64, 1024], F32, tag="s")
                    for c in (0, 512):
                        nc.tensor.matmul(rbbp[:, c:c + 512], bone[:],
                                         rb[:, c:c + 512],
                                         start=True, stop=True)
                    rbbs = asb.tile([64, 1024], BF16, tag="rbbs")
                    nc.vector.tensor_copy(rbbs[:], rbbp[:])
                    nc.vector.tensor_mul(
                        ctxT[qrow:qrow + 64, m * T + base:m * T + base + 1024],
                        ctx[0:64, :], rbbs[:])

                norm_pend = None
                for h in range(4):
                    sub, m = h % 2, h // 2
                    qrow = 64 * sub
                    for half in range(2):
                        base = 1024 * half
                        nstrips = 8 if half == 0 else 16
                        ctx = actx.tile([65, 1024], F32, tag="ctx")
                        pend = s_of(0, qrow, m, base)
                        if norm_pend is not None:
                            do_norm(*norm_pend)
                            norm_pend = None
                        for i in range(nstrips):
                            st, swin, r, wdt = pend
                            pt = aptp.tile([128, 1024], BF16, tag="pt")
                            nc.scalar.activation(pt[:, 128 * r:wdt],
                                                 st[:, 128 * r:wdt],
                                                 AF.Exp, scale=0.125)
                            if r > 0:
                                nc.vector.memset(pt[:, 0:128 * r], 0.0)
                            if 128 * i >= base:
                                nc.vector.tensor_mul(
                                    pt[:, 128 * r:128 * r + 128],
                                    pt[:, 128 * r:128 * r + 128], tri[:])
                            if i + 1 < nstrips:
                                pend = s_of(i + 1, qrow, m, base)
                            for j in range(swin // 512, 2):
                                last = 4 * j + 3 if half == 0 else 4 * j + 11
                                nc.tensor.matmul(
                                    ctx[:, 512 * j:512 * (j + 1)],
                                    vaug[:, 65 * i:65 * (i + 1)],
                                    pt[:, 512 * j - swin:512 * j - swin + 512],
                                    start=(i == 0), stop=(i == last))
                        norm_pend = (ctx, qrow, m, base)
                do_norm(*norm_pend)

            # ---------------- Stage 3: out-projection ----------------
            with tc.tile_pool(name="ops", bufs=6, space="PSUM") as ops, \
                 tc.tile_pool(name="osb", bufs=6) as osb:
                for tw in range(4):
                    for cb in range(16):
                        po = ops.tile([128, 512], F32, tag="po")
                        for mm in range(2):
                            nc.tensor.matmul(
                                po[:],
                                wo[:, T * mm + 128 * cb:T * mm + 128 * (cb + 1)],
                                ctxT[:, T * mm + 512 * tw:T * mm + 512 * (tw + 1)],
                                start=(mm == 0), stop=(mm == 1))
                        ob = osb.tile([128, 512], BF16, tag="ob")
                        if cb % 2 == 0:
                            nc.scalar.copy(ob[:], po[:])
                        else:
                            nc.vector.tensor_copy(ob[:], po[:])
                        nc.sync.dma_start(
                            out_d[128 * cb:128 * (cb + 1),
                                  512 * tw:512 * (tw + 1)], ob[:])

    nc.compile()
    return nc


def kernel(x, mask, cos, sin, W_query, W_key, W_value, W_out,
           q_norm_w, k_norm_w):
    global _PROG
    if _PROG is None:
        _PROG = _build_program()
    nc = _PROG

    x = np.asarray(x, np.float32)
    cos = np.asarray(cos, np.float32)
    sin = np.asarray(sin, np.float32)
    W_query = np.asarray(W_query, np.float32)
    W_key = np.asarray(W_key, np.float32)
    W_value = np.asarray(W_value, np.float32)
    W_out = np.asarray(W_out, np.float32)
    q_norm_w = np.asarray(q_norm_w, np.float32)
    k_norm_w = np.asarray(k_norm_w, np.float32)

    xt = np.ascontiguousarray(x[0].T).astype(BF)            # [C, T]

    # RoPE tables with the RMSNorm weights folded in:
    #   rope(q * w)[d] = q[d] w[d] cos[d] + sign[d] q[d^32] w[d^32] sin[d]
    cos1 = cos[:T].T.astype(np.float32)                     # [64, T]
    sin1 = sin[:T].T.astype(np.float32).copy()
    sgn = np.where((np.arange(D) % 64) < 32, -1.0, 1.0).astype(np.float32)
    dperm = np.arange(D) ^ 32
    cq1 = cos1 * q_norm_w[:, None]
    sq1 = sin1 * sgn[:, None] * q_norm_w[dperm][:, None]
    cosq = np.ascontiguousarray(np.concatenate([cq1, cq1], 0)).astype(BF)
    sinq = np.ascontiguousarray(np.concatenate([sq1, sq1], 0)).astype(BF)
    cosk = np.ascontiguousarray(cos1 * k_norm_w[:, None]).astype(BF)
    sink = np.ascontiguousarray(
        sin1 * sgn[:, None] * k_norm_w[dperm][:, None]).astype(BF)

    p = np.arange(128)[:, None]
    j = np.arange(128)[None, :]
    tri = np.where(p > j, 0.0, 1.0).astype(BF)              # S^T triangle

    sqo = np.zeros((128, 2), np.float32)
    sqo[0:64, 0] = 1.0
    sqo[64:128, 1] = 1.0
    perm = np.zeros((128, 128), np.float32)
    for b in range(2):
        for d_ in range(64):
            perm[64 * b + (d_ ^ 32), 64 * b + d_] = 1.0
    bsel = np.zeros((65, 320), np.float32)
    bsel[0, 0:64] = 1.0
    bsel[1, 64:128] = 1.0
    bsel[32, 128 + 0:128 + 64] = 1.0
    bsel[33, 128 + 64:128 + 128] = 1.0
    bsel[64, 256:320] = 1.0

    shared = {
        "xt": xt, "cosq": cosq, "sinq": sinq, "cosk": cosk, "sink": sink,
        "tri": tri, "sqo": sqo.astype(BF), "perm": perm.astype(BF),
        "id64": np.eye(64, dtype=np.float32).astype(BF),
        "onesv": np.ones((128, 16), np.float32).astype(BF),
        "bsel": bsel.astype(BF), "bone": np.ones((1, 64), np.float32).astype(BF),
    }
    in_maps = []
    for c in range(8):
        wqkv = np.concatenate(
            [W_query[DQ * c:DQ * (c + 1)],
             W_key[64 * c:64 * (c + 1)],
             W_value[64 * c:64 * (c + 1)]], axis=0).T       # [C, 384]
        wq_h = np.ascontiguousarray(
            wqkv.reshape(16, 128, 384).transpose(1, 0, 2).reshape(128, 6144)
        ).astype(BF)
        wo_sl = W_out[:, DQ * c:DQ * (c + 1)].T             # [256, C]
        wo_h = np.ascontiguousarray(
            wo_sl.reshape(2, 128, C).transpose(1, 0, 2).reshape(128, 2 * C)
        ).astype(BF)
        in_maps.append(dict(shared, wq=wq_h, wo=wo_h))

    res = run_bass_kernel_spmd(nc, in_maps, list(range(8)))
    acc = np.zeros((C, T), np.float32)
    for c in range(8):
        acc += res.results[c]["out"].astype(np.float32)
    return np.ascontiguousarray(acc.T)[None]


# revision 9
# speedup vs baseline: 2.3151x; 1.0986x over previous
"""GroupedQueryAttention TRN2 kernel: 8-way tensor-parallel over heads.

Sharding: core c gets query heads 4c..4c+3 (W_query rows 256c:256c+256),
KV head c (W_key/W_value rows 64c:64c+64), W_out columns 256c:256c+256.
x is replicated; each core computes a partial [C, T] output (transposed);
host transposes and sums.

All matmul operands are bf16 (1 PE cycle/row at any p-state and free size,
half the DMA bytes); PSUM accumulation stays f32.  Per-core dataflow:
  Stage 1 (per 512-col t-quarter, software-pipelined): xT streamed in,
    QKV projections (3 matmuls per 128-contraction chunk), PSUM->SBUF
    copies on ACT/DVE, RMS sumsq via PE ones-matmul + ACT sqrt + DVE
    recip, RoPE as rope_raw(q)*bcast(rinv) with the norm weights folded
    into per-dtype cos/sin tables (exact for any q/k_norm_w), v
    transposed into ones-augmented vaug via PE.
  Stage 2 attention per (head, 1024-col q-half): causal-trimmed S strips
    at 128 granularity, exp on ACT (scale=1/8 folded in) into bf16 P,
    triangle mask as bf16 multiply post-exp, A@V with ones-augmented V
    giving ctx + softmax sums in one accumulation; 1-ahead S pipeline
    against double-buffered PSUM.  Normalize via DVE recip + PE ones
    broadcast + DVE mul into bf16 ctxT.
  Stage 3 out-proj in [C, T] orientation (PSUM = [128 c-feat, 512 t]),
    copies alternate ACT/DVE, bf16 DMA out.
"""

import sys

sys.path.insert(0, "/opt/trn_rl_repo")

import numpy as np
import ml_dtypes

import concourse.bass as bass
import concourse.mybir as mybir
import concourse.tile as tile
from concourse import bacc
from concourse.bass_utils import run_bass_kernel_spmd

H, KV, D, EPS = 32, 8, 64, 1e-6
T = 2048
C = 2048
DQ = 256              # q out dims per core
NW = 512
F32 = mybir.dt.float32
BF16 = mybir.dt.bfloat16
AF = mybir.ActivationFunctionType
BF = ml_dtypes.bfloat16

_PROG = None


def _build_program():
    nc = bacc.Bacc("TRN2", target_bir_lowering=False, debug=False)

    xt_d = nc.declare_dram_parameter("xt", [C, T], BF16, isOutput=False)
    wq_d = nc.declare_dram_parameter("wq", [128, 16 * 384], BF16, isOutput=False)
    wo_d = nc.declare_dram_parameter("wo", [128, 2 * T], BF16, isOutput=False)
    cosq_d = nc.declare_dram_parameter("cosq", [128, T], BF16, isOutput=False)
    sinq_d = nc.declare_dram_parameter("sinq", [128, T], BF16, isOutput=False)
    cosk_d = nc.declare_dram_parameter("cosk", [64, T], BF16, isOutput=False)
    sink_d = nc.declare_dram_parameter("sink", [64, T], BF16, isOutput=False)
    tri_d = nc.declare_dram_parameter("tri", [128, 128], BF16, isOutput=False)
    sqo_d = nc.declare_dram_parameter("sqo", [128, 2], BF16, isOutput=False)
    perm_d = nc.declare_dram_parameter("perm", [128, 128], BF16, isOutput=False)
    id64_d = nc.declare_dram_parameter("id64", [64, 64], BF16, isOutput=False)
    onesv_d = nc.declare_dram_parameter("onesv", [128, 16], BF16, isOutput=False)
    bsel_d = nc.declare_dram_parameter("bsel", [65, 320], BF16, isOutput=False)
    bone_d = nc.declare_dram_parameter("bone", [1, 64], BF16, isOutput=False)
    out_d = nc.declare_dram_parameter("out", [C, T], BF16, isOutput=True)

    with tile.TileContext(nc) as tc:
        with tc.tile_pool(name="persist", bufs=1) as pp:
            qT = pp.tile([128, 2 * T], BF16, tag="qT")
            kkT = pp.tile([128, T], BF16, tag="kkT")
            vaug = pp.tile([128, 16 * 65], BF16, tag="vaug")
            ctxT = pp.tile([128, 2 * T], BF16, tag="ctxT")
            cosq = pp.tile([128, T], BF16, tag="cosq")
            sinq = pp.tile([128, T], BF16, tag="sinq")
            cosk = pp.tile([64, T], BF16, tag="cosk")
            sink = pp.tile([64, T], BF16, tag="sink")
            wq = pp.tile([128, 16 * 384], BF16, tag="wq")
            wo = pp.tile([128, 2 * T], BF16, tag="wo")
            tri = pp.tile([128, 128], BF16, tag="tri")
            sqo = pp.tile([128, 2], BF16, tag="sqo")
            perm = pp.tile([128, 128], BF16, tag="perm")
            id64 = pp.tile([64, 64], BF16, tag="id64")
            bsel = pp.tile([65, 320], BF16, tag="bsel")
            bone = pp.tile([1, 64], BF16, tag="bone")
            epsb = pp.tile([65, 1], F32, tag="epsb")
            nc.vector.memset(epsb[:], float(EPS))
            # first ACT op is a Sqrt so the initial table load picks the
            # sqrt set (stage-1 Copy lives there too); the only other load
            # is the exp set at attention (tail Copy reuses it).
            dsq = pp.tile([1, 1], F32, tag="dsq")
            nc.scalar.activation(dsq[:], epsb[0:1, :], AF.Sqrt,
                                 bias=epsb[0:1, :], scale=1.0)

            # ---------------- Stage 1: QKV + RMSNorm + RoPE ----------------
            with tc.tile_pool(name="s1x", bufs=2) as s1x, \
                 tc.tile_pool(name="s1s", bufs=2) as s1s, \
                 tc.tile_pool(name="s1pq", bufs=2, space="PSUM") as s1pq, \
                 tc.tile_pool(name="s1sm", bufs=2, space="PSUM") as s1sm:

                def xload(qtr):
                    xq = s1x.tile([128, 16 * NW], BF16, tag="xq")
                    src = xt_d[:, qtr * NW:(qtr + 1) * NW].rearrange(
                        "(c p) t -> p c t", p=128)
                    dst = xq[:].rearrange("p (c t) -> p c t", t=NW)
                    for g_ in range(4):
                        nc.sync.dma_start(dst[:, 4 * g_:4 * (g_ + 1), :],
                                          src[:, 4 * g_:4 * (g_ + 1), :])
                    return xq

                def qkv(qtr, xq=None):
                    if xq is None:
                        xq = xload(qtr)
                    pq0 = s1pq.tile([128, NW], F32, tag="pq0")
                    pq1 = s1pq.tile([128, NW], F32, tag="pq1")
                    pkv = s1pq.tile([128, NW], F32, tag="pkv")
                    for ci in range(16):
                        st, sp = ci == 0, ci == 15
                        xc = xq[:, NW * ci:NW * (ci + 1)]
                        nc.tensor.matmul(pq0[:], wq[:, 384 * ci:384 * ci + 128],
                                         xc, start=st, stop=sp)
                        nc.tensor.matmul(pq1[:],
                                         wq[:, 384 * ci + 128:384 * ci + 256],
                                         xc, start=st, stop=sp)
                        nc.tensor.matmul(pkv[:],
                                         wq[:, 384 * ci + 256:384 * ci + 384],
                                         xc, start=st, stop=sp)
                    return pq0, pq1, pkv

                def post(qtr, pq0, pq1, pkv):
                    w0 = qtr * NW
                    qraw0 = s1s.tile([128, NW], BF16, tag="qraw0")
                    qraw1 = s1s.tile([128, NW], BF16, tag="qraw1")
                    kraw = s1s.tile([64, NW], BF16, tag="kraw")
                    vraw = s1s.tile([64, NW], BF16, tag="vraw")
                    nc.scalar.copy(qraw0[:], pq0[:])
                    nc.scalar.copy(qraw1[:], pq1[:])
                    nc.vector.tensor_copy(kraw[:], pkv[0:64, :])
                    nc.vector.tensor_copy(vraw[:], pkv[64:128, :])
                    # PE: perm matmuls first (only need raw copies)
                    ppm0 = s1sm.tile([128, NW], F32, tag="sm")
                    nc.tensor.matmul(ppm0[:], perm[:], qraw0[:],
                                     start=True, stop=True)
                    ppm1 = s1sm.tile([128, NW], F32, tag="sm")
                    nc.tensor.matmul(ppm1[:], perm[:], qraw1[:],
                                     start=True, stop=True)
                    ppk = s1sm.tile([128, NW], F32, tag="sm")
                    nc.tensor.matmul(ppk[0:64, :], perm[0:64, 0:64], kraw[:],
                                     start=True, stop=True)
                    # RMS sumsq
                    t20 = s1s.tile([128, NW], BF16, tag="t20")
                    t21 = s1s.tile([128, NW], BF16, tag="t21")
                    t2k = s1s.tile([64, NW], BF16, tag="t2k")
                    nc.vector.tensor_mul(t20[:], qraw0[:], qraw0[:])
                    nc.vector.tensor_mul(t21[:], qraw1[:], qraw1[:])
                    nc.vector.tensor_mul(t2k[:], kraw[:], kraw[:])
                    srow = s1sm.tile([65, NW], F32, tag="sm")
                    nc.tensor.matmul(srow[0:2, :], sqo[:, 0:2], t20[:],
                                     start=True, stop=True)
                    nc.tensor.matmul(srow[32:34, :], sqo[:, 0:2], t21[:],
                                     start=True, stop=True)
                    nc.tensor.matmul(srow[64:65, :], sqo[0:64, 0:1], t2k[:],
                                     start=True, stop=True)
                    # RoPE partials that don't need rinv
                    tm1_0 = s1s.tile([128, NW], BF16, tag="tm1_0")
                    tm1_1 = s1s.tile([128, NW], BF16, tag="tm1_1")
                    tk1 = s1s.tile([64, NW], BF16, tag="tk1")
                    nc.vector.tensor_mul(tm1_0[:], qraw0[:], cosq[:, w0:w0 + NW])
                    nc.vector.tensor_mul(tm1_1[:], qraw1[:], cosq[:, w0:w0 + NW])
                    nc.vector.tensor_mul(tk1[:], kraw[:], cosk[:, w0:w0 + NW])
                    tsum0 = s1s.tile([128, NW], BF16, tag="tsum0")
                    tsum1 = s1s.tile([128, NW], BF16, tag="tsum1")
                    tks = s1s.tile([64, NW], BF16, tag="tks")
                    nc.vector.tensor_mul(tsum0[:], ppm0[:], sinq[:, w0:w0 + NW])
                    nc.vector.tensor_add(tsum0[:], tsum0[:], tm1_0[:])
                    nc.vector.tensor_mul(tsum1[:], ppm1[:], sinq[:, w0:w0 + NW])
                    nc.vector.tensor_add(tsum1[:], tsum1[:], tm1_1[:])
                    nc.vector.tensor_mul(tks[:], ppk[0:64, :], sink[:, w0:w0 + NW])
                    nc.vector.tensor_add(tks[:], tks[:], tk1[:])
                    # v -> vaug (transposed, ones-augmented)
                    for j in range(4):
                        ii = (w0 // 128) + j
                        pv = s1sm.tile([128, 64], BF16, tag="sm")
                        nc.tensor.transpose(pv[:], vraw[:, 128 * j:128 * (j + 1)],
                                            id64[:])
                        nc.vector.tensor_copy(vaug[:, 65 * ii:65 * ii + 64],
                                              pv[:])
                    # sqrt -> recip -> broadcast
                    rms5 = s1s.tile([65, NW], F32, tag="rms5")
                    for lo, hi in ((0, 2), (32, 34), (64, 65)):
                        nc.scalar.activation(rms5[lo:hi, :], srow[lo:hi, :],
                                             AF.Sqrt, bias=epsb[lo:hi, :],
                                             scale=1.0 / 64)
                    rb5 = s1s.tile([65, NW], BF16, tag="rb5")
                    with nc.allow_low_precision(reason="rmsnorm recip bf16"):
                        for lo, hi in ((0, 2), (32, 34), (64, 65)):
                            nc.vector.reciprocal(rb5[lo:hi, :], rms5[lo:hi, :])
                    bbs = []
                    for g, (plo, phi, lo, hi, npart) in enumerate(
                            ((0, 2, 0, 128, 128), (32, 34, 128, 256, 128),
                             (64, 65, 256, 320, 64))):
                        bb = s1sm.tile([128, NW], F32, tag="sm")
                        nc.tensor.matmul(bb[0:npart, :], bsel[plo:phi, lo:hi],
                                         rb5[plo:phi, :], start=True, stop=True)
                        bs = s1s.tile([128, NW], BF16, tag=f"bbs{g}")
                        nc.scalar.copy(bs[0:npart, :], bb[0:npart, :])
                        bbs.append(bs)
                    nc.vector.tensor_mul(qT[:, 0 * T + w0:0 * T + w0 + NW],
                                         tsum0[:], bbs[0][:])
                    nc.vector.tensor_mul(qT[:, 1 * T + w0:1 * T + w0 + NW],
                                         tsum1[:], bbs[1][:])
                    nc.vector.tensor_mul(kkT[0:64, w0:w0 + NW], tks[:],
                                         bbs[2][0:64, :])
                    nc.vector.tensor_copy(kkT[64:128, w0:w0 + NW],
                                          kkT[0:64, w0:w0 + NW])

                xq0 = s1x.tile([128, 16 * NW], BF16, tag="xq")
                src0 = xt_d[:, 0:NW].rearrange("(c p) t -> p c t", p=128)
                dst0 = xq0[:].rearrange("p (c t) -> p c t", t=NW)
                for g_ in range(4):
                    nc.sync.dma_start(wq[:, 1536 * g_:1536 * (g_ + 1)],
                                      wq_d[:, 1536 * g_:1536 * (g_ + 1)])
                    nc.sync.dma_start(dst0[:, 4 * g_:4 * (g_ + 1), :],
                                      src0[:, 4 * g_:4 * (g_ + 1), :])
                cur = qkv(0, xq0)
                xq1 = xload(1)
                for t_, d_ in ((cosq, cosq_d), (sinq, sinq_d), (cosk, cosk_d),
                               (sink, sink_d), (sqo, sqo_d), (perm, perm_d),
                               (id64, id64_d), (bsel, bsel_d), (tri, tri_d),
                               (bone, bone_d)):
                    nc.sync.dma_start(t_[:], d_[:])
                nc.sync.dma_start(
                    vaug[:].rearrange("p (i c) -> p i c", c=65)[:, :, 64:65],
                    onesv_d[:].rearrange("p (i c) -> p i c", c=1),
                )
                prev = cur
                cur = qkv(1, xq1)
                post(0, *prev)
                prev = cur
                cur = qkv(2)
                post(1, *prev)
                prev = cur
                cur = qkv(3)
                post(2, *prev)
                nc.sync.dma_start(wo[:], wo_d[:])
                post(3, *cur)

            # ------- Stage 2+3: attention (window-outer) + fused out-proj ----
            # ctx = [65, 512] per (head, 512-col q-window); out-proj for
            # window w interleaves into window w+1's attention stream.
            with tc.tile_pool(name="actx", bufs=2, space="PSUM") as actx, \
                 tc.tile_pool(name="asp", bufs=3, space="PSUM") as asp, \
                 tc.tile_pool(name="ops", bufs=3, space="PSUM") as ops, \
                 tc.tile_pool(name="aptp", bufs=3) as aptp, \
                 tc.tile_pool(name="asb", bufs=2) as asb, \
                 tc.tile_pool(name="osb", bufs=4) as osb:

                def s_of(i, qrow, m, w):
                    col0 = max(128 * i - 512 * w, 0)     # window-relative
                    st = asp.tile([128, 512], F32, tag="s")
                    nc.tensor.matmul(
                        st[:, col0:512],
                        kkT[qrow:qrow + 64, 128 * i:128 * (i + 1)],
                        qT[qrow:qrow + 64,
                           m * T + 512 * w + col0:m * T + 512 * (w + 1)],
                        start=True, stop=True)
                    return st, col0

                def do_norm(ctx, qrow, m, w):
                    rb = asb.tile([1, 512], BF16, tag="rb")
                    with nc.allow_low_precision(reason="softmax recip"):
                        nc.vector.reciprocal(rb[:], ctx[64:65, :])
                    rbbp = asp.tile([64, 512], F32, tag="s")
                    nc.tensor.matmul(rbbp[:], bone[:], rb[:],
                                     start=True, stop=True)
                    rbbs = asb.tile([64, 512], BF16, tag="rbbs")
                    nc.vector.tensor_copy(rbbs[:], rbbp[:])
                    nc.vector.tensor_mul(
                        ctxT[qrow:qrow + 64, m * T + 512 * w:m * T + 512 * (w + 1)],
                        ctx[0:64, :], rbbs[:])

                def outproj(tw, cbs):
                    for cb in cbs:
                        po = ops.tile([128, 512], F32, tag="po")
                        for mm in range(2):
                            nc.tensor.matmul(
                                po[:],
                                wo[:, T * mm + 128 * cb:T * mm + 128 * (cb + 1)],
                                ctxT[:, T * mm + 512 * tw:T * mm + 512 * (tw + 1)],
                                start=(mm == 0), stop=(mm == 1))
                        ob = osb.tile([128, 512], BF16, tag="ob")
                        if cb % 2 == 0:
                            nc.scalar.copy(ob[:], po[:])
                        else:
                            nc.vector.tensor_copy(ob[:], po[:])
                        nc.sync.dma_start(
                            out_d[128 * cb:128 * (cb + 1),
                                  512 * tw:512 * (tw + 1)], ob[:])

                norm_pend = None
                for w in range(4):
                    for h in range(4):
                        sub, m = h % 2, h // 2
                        qrow = 64 * sub
                        nstrips = 4 * w + 4
                        ctx = actx.tile([65, 512], F32, tag="ctx")
                        pend = s_of(0, qrow, m, w)
                        if norm_pend is not None:
                            do_norm(*norm_pend)
                            norm_pend = None
                        for i in range(nstrips):
                            st, col0 = pend
                            pt = aptp.tile([128, 512], BF16, tag="pt")
                            nc.scalar.activation(pt[:, col0:512], st[:, col0:512],
                                                 AF.Exp, scale=0.125)
                            if col0 > 0:
                                nc.vector.memset(pt[:, 0:col0], 0.0)
                            if i >= 4 * w:
                                nc.vector.tensor_mul(
                                    pt[:, col0:col0 + 128],
                                    pt[:, col0:col0 + 128], tri[:])
                            if i + 1 < nstrips:
                                pend = s_of(i + 1, qrow, m, w)
                            nc.tensor.matmul(
                                ctx[:], vaug[:, 65 * i:65 * (i + 1)], pt[:],
                                start=(i == 0), stop=(i == nstrips - 1))
                        norm_pend = (ctx, qrow, m, w)
                        if w > 0:
                            outproj(w - 1, range(4 * h, 4 * h + 4))
                do_norm(*norm_pend)
                outproj(3, range(16))

    nc.compile()
    return nc


def kernel(x, mask, cos, sin, W_query, W_key, W_value, W_out,
           q_norm_w, k_norm_w):
    global _PROG
    if _PROG is None:
        _PROG = _build_program()
    nc = _PROG

    x = np.asarray(x, np.float32)
    cos = np.asarray(cos, np.float32)
    sin = np.asarray(sin, np.float32)
    W_query = np.asarray(W_query, np.float32)
    W_key = np.asarray(W_key, np.float32)
    W_value = np.asarray(W_value, np.float32)
    W_out = np.asarray(W_out, np.float32)
    q_norm_w = np.asarray(q_norm_w, np.float32)
    k_norm_w = np.asarray(k_norm_w, np.float32)

    xt = np.ascontiguousarray(x[0].T).astype(BF)            # [C, T]

    # RoPE tables with the RMSNorm weights folded in:
    #   rope(q * w)[d] = q[d] w[d] cos[d] + sign[d] q[d^32] w[d^32] sin[d]
    cos1 = cos[:T].T.astype(np.float32)                     # [64, T]
    sin1 = sin[:T].T.astype(np.float32).copy()
    sgn = np.where((np.arange(D) % 64) < 32, -1.0, 1.0).astype(np.float32)
    dperm = np.arange(D) ^ 32
    cq1 = cos1 * q_norm_w[:, None]
    sq1 = sin1 * sgn[:, None] * q_norm_w[dperm][:, None]
    cosq = np.ascontiguousarray(np.concatenate([cq1, cq1], 0)).astype(BF)
    sinq = np.ascontiguousarray(np.concatenate([sq1, sq1], 0)).astype(BF)
    cosk = np.ascontiguousarray(cos1 * k_norm_w[:, None]).astype(BF)
    sink = np.ascontiguousarray(
        sin1 * sgn[:, None] * k_norm_w[dperm][:, None]).astype(BF)

    p = np.arange(128)[:, None]
    j = np.arange(128)[None, :]
    tri = np.where(p > j, 0.0, 1.0).astype(BF)              # S^T triangle

    sqo = np.zeros((128, 2), np.float32)
    sqo[0:64, 0] = 1.0
    sqo[64:128, 1] = 1.0
    perm = np.zeros((128, 128), np.float32)
    for b in range(2):
        for d_ in range(64):
            perm[64 * b + (d_ ^ 32), 64 * b + d_] = 1.0
    bsel = np.zeros((65, 320), np.float32)
    bsel[0, 0:64] = 1.0
    bsel[1, 64:128] = 1.0
    bsel[32, 128 + 0:128 + 64] = 1.0
    bsel[33, 128 + 64:128 + 128] = 1.0
    bsel[64, 256:320] = 1.0

    shared = {
        "xt": xt, "cosq": cosq, "sinq": sinq, "cosk": cosk, "sink": sink,
        "tri": tri, "sqo": sqo.astype(BF), "perm": perm.astype(BF),
        "id64": np.eye(64, dtype=np.float32).astype(BF),
        "onesv": np.ones((128, 16), np.float32).astype(BF),
        "bsel": bsel.astype(BF), "bone": np.ones((1, 64), np.float32).astype(BF),
    }
    in_maps = []
    for c in range(8):
        wqkv = np.concatenate(
            [W_query[DQ * c:DQ * (c + 1)],
             W_key[64 * c:64 * (c + 1)],
             W_value[64 * c:64 * (c + 1)]], axis=0).T       # [C, 384]
        wq_h = np.ascontiguousarray(
            wqkv.reshape(16, 128, 384).transpose(1, 0, 2).reshape(128, 6144)
        ).astype(BF)
        wo_sl = W_out[:, DQ * c:DQ * (c + 1)].T             # [256, C]
        wo_h = np.ascontiguousarray(
            wo_sl.reshape(2, 128, C).transpose(1, 0, 2).reshape(128, 2 * C)
        ).astype(BF)
        in_maps.append(dict(shared, wq=wq_h, wo=wo_h))

    res = run_bass_kernel_spmd(nc, in_maps, list(range(8)))
    acc = np.zeros((C, T), np.float32)
    for c in range(8):
        acc += res.results[c]["out"].astype(np.float32)
    return np.ascontiguousarray(acc.T)[None]
